# revision 1
# baseline (speedup 1.0000x reference)
"""LocalWindowAttention (3x3 windows, B=16, 96x96, C=256, 4 heads) on 8
Trainium2 NeuronCores via Bass/Tile. Pure data parallel: 2 images per core.

Self-contained: builds the per-core Bass program, shards the batch, runs
SPMD on cores 0-7, gathers the full output.
"""

import numpy as np
import ml_dtypes

import concourse.bass as bass
import concourse.bacc as bacc
import concourse.tile as tile
from concourse import mybir
from concourse.bass_utils import run_bass_kernel_spmd

F32 = mybir.dt.float32
BF16 = mybir.dt.bfloat16

B = 16
NCORES = 8
IMG = B // NCORES          # images per core
C = 256
NH = 4
HD = 64
WS = 3
GRID = 96                  # H = W = 96
S = GRID // WS             # 32 window-rows ("strips") per image
NT = GRID * GRID           # tokens per image
SCALE = HD ** -0.5
BLOCKS = [(0, 14), (14, 14), (28, 4)]  # (wcol0, nwin); block tokens = 9*nwin


def _build(nc, ns=S, img=IMG, reps=1):
    nt = ns * 288
    x = nc.declare_dram_parameter("x", [img, nt, C], F32, isOutput=False).ap()
    wqkvT = nc.declare_dram_parameter("wqkvT", [128, 2, 768], BF16, isOutput=False).ap()
    wprojT = nc.declare_dram_parameter("wprojT", [128, 2, 256], BF16, isOutput=False).ap()
    maskc = nc.declare_dram_parameter("maskc", [128, 256], BF16, isOutput=False).ap()
    identc = nc.declare_dram_parameter("identc", [128, 128], BF16, isOutput=False).ap()
    onesc = nc.declare_dram_parameter("onesc", [128, 128], BF16, isOutput=False).ap()
    y = nc.declare_dram_parameter("y", [img, nt, C], F32, isOutput=True).ap()

    # [img, wrow, r, col, chan]; block b covers cols 42b : 42b+3*nwin
    xv = x.rearrange("b (wr r col) ch -> b wr r col ch", r=WS, col=GRID)
    yv = y.rearrange("b (wr r col) ch -> b wr r col ch", r=WS, col=GRID)

    with tile.TileContext(nc) as tc:
        with (
            tc.tile_pool(name="const", bufs=1) as constp,
            tc.tile_pool(name="sb", bufs=1) as sb,
            tc.tile_pool(name="ps", bufs=1, space="PSUM") as ps,
        ):
            wq_sb = constp.tile([128, 2, 768], BF16)
            nc.sync.dma_start(out=wq_sb[:], in_=wqkvT[:])
            wp_sb = constp.tile([128, 2, 256], BF16)
            nc.sync.dma_start(out=wp_sb[:], in_=wprojT[:])
            mask_sb = constp.tile([128, 256], BF16)
            nc.sync.dma_start(out=mask_sb[:], in_=maskc[:])
            ident_sb = constp.tile([128, 128], BF16)
            nc.sync.dma_start(out=ident_sb[:], in_=identc[:])
            ones_sb = constp.tile([128, 128], BF16)
            nc.sync.dma_start(out=ones_sb[:], in_=onesc[:])

            def _all():
                for b_ in range(img):
                    for w in range(ns):
                        _strip(nc, sb, ps, xv, yv, b_, w,
                               wq_sb, wp_sb, mask_sb, ident_sb, ones_sb)

            if reps == 1:
                _all()
            else:
                with tc.For_i(0, reps, 1):
                    _all()
    return nc


def _strip(nc, sb, ps, xv, yv, img, w, wq_sb, wp_sb, mask_sb, ident_sb, ones_sb):
    # ---- load x (strip = one window-row = 288 tokens), per-block DMAs ----
    x_sb = sb.tile([128, 3, C], F32, tag="x", bufs=3)
    for b, (wc0, nb) in enumerate(BLOCKS):
        nc.sync.dma_start(
            out=x_sb[0 : nb * 9, b, :],
            in_=xv[img, w, :, 3 * wc0 : 3 * (wc0 + nb), :],
        )

    # ---- cast to bf16 (gpsimd) ----
    x_bf = sb.tile([128, 3, C], BF16, tag="xbf", bufs=2)
    for b, (wc0, nb) in enumerate(BLOCKS):
        kb = nb * 9
        nc.gpsimd.tensor_copy(out=x_bf[0:kb, b, :], in_=x_sb[0:kb, b, :])

    # ---- x^T via PE transpose: [128ch, 288tok] in 2 chan-chunks ----
    xT_ps = ps.tile([128, 2, 288], BF16, tag="xT", bufs=1)
    for b, (wc0, nb) in enumerate(BLOCKS):
        kb = nb * 9
        for cc in range(2):
            nc.tensor.transpose(
                out=xT_ps[:, cc, 126 * b : 126 * b + kb],
                in_=x_bf[0:kb, b, 128 * cc : 128 * cc + 128],
                identity=ident_sb[0:kb, 0:kb],
            )
    xT_sb = sb.tile([128, 2, 288], BF16, tag="xTs", bufs=2)
    nc.vector.tensor_copy(out=xT_sb[:], in_=xT_ps[:])

    # ---- q^T, k^T channel-major; chunk mc holds heads (2mc, 2mc+1) ----
    qkT_ps = []
    for t, base in ((0, 0), (1, 256)):  # t=0 -> q, t=1 -> k
        chunks = []
        for mc in range(2):
            qp = ps.tile([128, 288], F32, tag="qk", bufs=2)
            for kc in range(2):
                nc.tensor.matmul(
                    out=qp[:],
                    lhsT=wq_sb[:, kc, base + 128 * mc : base + 128 * mc + 128],
                    rhs=xT_sb[:, kc, :],
                    start=(kc == 0),
                    stop=(kc == 1),
                )
            chunks.append(qp)
        qkT_ps.append(chunks)
    qT_sb = sb.tile([128, 2, 288], BF16, tag="qTs", bufs=2)
    kT_sb = sb.tile([128, 2, 288], BF16, tag="kTs", bufs=2)
    for mc in range(2):
        nc.vector.tensor_copy(out=qT_sb[:, mc, :], in_=qkT_ps[0][mc][:])
        nc.scalar.copy(out=kT_sb[:, mc, :], in_=qkT_ps[1][mc][:])

    # ---- v token-major: per block [kb, 256] ----
    v_sb = sb.tile([128, 3, C], BF16, tag="vs", bufs=2)
    for b, (wc0, nb) in enumerate(BLOCKS):
        kb = nb * 9
        vp = ps.tile([128, 512], F32, tag="sm", bufs=3)
        for kc in range(2):
            nc.tensor.matmul(
                out=vp[0:kb, 0:256],
                lhsT=xT_sb[:, kc, 126 * b : 126 * b + kb],
                rhs=wq_sb[:, kc, 512:768],
                start=(kc == 0),
                stop=(kc == 1),
            )
        nc.scalar.copy(out=v_sb[0:kb, b, :], in_=vp[0:kb, 0:256])

    # ---- QK^T logits^T per block/head: [k, q] ----
    # expm head order within a block: [h0, h2, h1, h3] — even-row-group MMs
    # land in bank E, odd-row-group in bank O (same-bank mixed row groups
    # are an unrecoverable HW fault).
    expm = sb.tile([126, 12, 126], BF16, tag="expm", bufs=2)
    for b, (wc0, nb) in enumerate(BLOCKS):
        kb = nb * 9
        # row-group hh writes its own PSUM bank (free-offset 512*hh):
        # mixing row groups within one bank is an unrecoverable HW fault.
        aL = ps.tile([128, 2, 512], F32, tag="att", bufs=1)
        for mc in range(2):
            for hh in range(2):
                p0 = 64 * hh
                nc.tensor.matmul(
                    out=aL[0:kb, hh, 126 * mc : 126 * mc + kb],
                    lhsT=kT_sb[p0 : p0 + 64, mc, 126 * b : 126 * b + kb],
                    rhs=qT_sb[p0 : p0 + 64, mc, 126 * b : 126 * b + kb],
                    start=True,
                    stop=True,
                )
        # one exp op over both banks; head order per block: h0, h2, h1, h3
        ein = bass.AP(tensor=aL.tensor, offset=aL.offset,
                      ap=[[aL.ap[0][0], kb], [512, 2], [126, 2], [1, kb]])
        nc.scalar.activation(
            out=expm[0:kb, 4 * b : 4 * b + 4, 0:kb], in_=ein,
            func=mybir.ActivationFunctionType.Exp, scale=SCALE)

    # ---- mask (DVE): expm *= blockdiag(9) ----
    m = mask_sb[0:126, 0:126]
    mb = bass.AP(tensor=m.tensor, offset=m.offset,
                 ap=[m.ap[0], [0, 8], m.ap[1]])
    nc.vector.tensor_mul(
        out=expm[:, 0:8, :], in0=expm[:, 0:8, :], in1=mb)
    m2 = mask_sb[0:36, 128 : 128 + 36]
    mb2 = bass.AP(tensor=m2.tensor, offset=m2.offset,
                  ap=[m2.ap[0], [0, 4], m2.ap[1]])
    nc.vector.tensor_mul(
        out=expm[0:36, 8:12, 0:36], in0=expm[0:36, 8:12, 0:36], in1=mb2)

    # ---- denominators broadcast over 64-row groups via ones-matmul ----
    rbc = []
    for T in range(2):
        dp = ps.tile([128, 512], F32, tag="sm", bufs=3)
        for hh in range(2):
            h = 2 * T + hh
            hc = (h % 2) * 2 + h // 2
            e01 = expm[:, hc, :]
            e01 = bass.AP(tensor=e01.tensor, offset=e01.offset,
                          ap=[e01.ap[0], [504, 2], [1, 126]])
            nc.tensor.matmul(
                out=dp[64 * hh : 64 * hh + 64, 0:252],
                lhsT=ones_sb[0:126, 0:64],
                rhs=e01,
                start=True, stop=True,
                tile_position=(0, 64 * hh),
            )
            nc.tensor.matmul(
                out=dp[64 * hh : 64 * hh + 64, 252:288],
                lhsT=ones_sb[0:36, 0:64],
                rhs=expm[0:36, 8 + hc, 0:36],
                start=True, stop=True,
                tile_position=(0, 64 * hh),
            )
        r = sb.tile([128, 288], F32, tag="rbc", bufs=2)
        nc.vector.reciprocal_approx_fast(out=r[:, 0:288], in_=dp[:, 0:288])
        rbc.append(r)

    # ---- AV: unnormalized channel-major ao; normalize during evac ----
    ao_sb = sb.tile([128, 2, 288], BF16, tag="aos", bufs=2)
    for T in range(2):
        ap_ = ps.tile([128, 512], F32, tag="sm", bufs=3)
        for hh in range(2):
            h = 2 * T + hh
            hc = (h % 2) * 2 + h // 2
            for b, (wc0, nb) in enumerate(BLOCKS):
                kb = nb * 9
                nc.tensor.matmul(
                    out=ap_[64 * hh : 64 * hh + 64, 126 * b : 126 * b + kb],
                    lhsT=v_sb[0:kb, b, 64 * h : 64 * h + 64],
                    rhs=expm[0:kb, 4 * b + hc, 0:kb],
                    start=True, stop=True,
                    tile_position=(0, 64 * hh),
                )
        nc.vector.tensor_mul(out=ao_sb[:, T, 0:288], in0=ap_[:, 0:288],
                             in1=rbc[T][:, 0:288])

    # ---- proj + output evac + scatter ----
    out_sb = sb.tile([128, 3, C], F32, tag="outs", bufs=2)
    for b, (wc0, nb) in enumerate(BLOCKS):
        kb = nb * 9
        op = ps.tile([128, 512], F32, tag="sm", bufs=3)
        for T in range(2):
            nc.tensor.matmul(
                out=op[0:kb, 0:256],
                lhsT=ao_sb[:, T, 126 * b : 126 * b + kb],
                rhs=wp_sb[:, T, :],
                start=(T == 0),
                stop=(T == 1),
            )
        nc.scalar.copy(out=out_sb[0:kb, b, :], in_=op[0:kb, 0:256])
        nc.sync.dma_start(
            out=yv[img, w, :, 3 * wc0 : 3 * (wc0 + nb), :],
            in_=out_sb[0 : nb * 9, b, :],
        )


def _make_consts():
    bf16 = ml_dtypes.bfloat16
    mask = np.zeros((128, 256), np.float32)
    for p in range(126):
        for q in range(126):
            if (p % 42) // 3 == (q % 42) // 3:
                mask[p, q] = 1.0
    for p in range(36):
        for q in range(36):
            if (p % 12) // 3 == (q % 12) // 3:
                mask[p, 128 + q] = 1.0
    return {
        "maskc": mask.astype(bf16),
        "identc": np.eye(128, dtype=np.float32).astype(bf16),
        "onesc": np.ones((128, 128), np.float32).astype(bf16),
    }


_NC_CACHE = {}


def _get_nc():
    if "nc" not in _NC_CACHE:
        nc = bacc.Bacc("TRN2", target_bir_lowering=False, debug=False,
                       num_devices=NCORES)
        _build(nc)
        nc.compile()
        _NC_CACHE["nc"] = nc
    return _NC_CACHE["nc"]


def _in_maps(x, Wqkv, Wproj):
    bf16 = ml_dtypes.bfloat16
    consts = _make_consts()
    consts["wqkvT"] = np.ascontiguousarray(
        np.asarray(Wqkv, np.float32).T.reshape(2, 128, 768).transpose(1, 0, 2)
    ).astype(bf16)
    consts["wprojT"] = np.ascontiguousarray(
        np.asarray(Wproj, np.float32).T.reshape(2, 128, 256).transpose(1, 0, 2)
    ).astype(bf16)
    x = np.asarray(x, np.float32)
    return [{"x": x[IMG * c : IMG * c + IMG], **consts} for c in range(NCORES)]


def kernel(x, Wqkv, Wproj, H, W):
    assert int(H) == GRID and int(W) == GRID
    nc = _get_nc()
    res = run_bass_kernel_spmd(nc, _in_maps(x, Wqkv, Wproj), list(range(NCORES)))
    out = np.concatenate([res.results[c]["y"] for c in range(NCORES)], axis=0)
    return np.ascontiguousarray(out.reshape(B, NT, C))



# revision 22
# speedup vs baseline: 2.7518x; 2.7518x over previous
"""LocalWindowAttention (3x3 windows, B=16, 96x96, C=256, 4 heads) on 8
Trainium2 NeuronCores via Bass/Tile. Pure data parallel: 2 images per core.

v2 pipeline (per strip = one window-row = 288 tokens, 32 windows):
  - host pre-permutes x into padded blocks [3, 128, 256] bf16 per strip so
    each strip is ONE input DMA; output likewise one bf16 DMA per strip,
    inverse-permuted + cast to f32 on host.
  - PE: x^T transpose -> q^T,k^T (chan-major) -> v (tok-major) -> QK^T
    (block-diag, 14/14/4 windows) -> AV tok-major with a ones-column per
    head appended to v so the softmax denominator falls out of the same
    matmul -> ao^T transpose -> proj.
  - softmax: exp on ScalarE, window mask multiply on GpSimd, reciprocal +
    normalize on DVE (free-dim broadcast of 1/denom).
  - 3-stage software-pipeline skew across strips: emit S2(t-1), S1(t),
    S3(t-2) so each engine's in-order queue never stalls on same-strip
    cross-engine deps.
"""

import os
import numpy as np
import ml_dtypes

_DBG = set(os.environ.get("KDBG", "").split(","))

import concourse.bass as bass
import concourse.bacc as bacc
import concourse.tile as tile
from concourse import mybir
from concourse.bass_utils import run_bass_kernel_spmd

F32 = mybir.dt.float32
BF16 = mybir.dt.bfloat16

B = 16
NCORES = 8
IMG = B // NCORES          # images per core
C = 256
NH = 4
HD = 64
WS = 3
GRID = 96                  # H = W = 96
S = GRID // WS             # 32 window-rows ("strips") per image
NT = GRID * GRID           # tokens per image
SCALE = HD ** -0.5
BLOCKS = [(0, 14), (14, 14), (28, 4)]  # (wcol0, nwin); block tokens = 9*nwin
KBS = [126, 126, 36]


def _ap(t, dims):
    return bass.AP(tensor=t.tensor, offset=t.offset, ap=dims)


def _build(nc, ns=S, img=IMG, reps=1):
    x = nc.declare_dram_parameter("x", [img, ns, 3, 128, C], BF16, isOutput=False).ap()
    wqkvT = nc.declare_dram_parameter("wqkvT", [128, 2, 768], BF16, isOutput=False).ap()
    wprojT = nc.declare_dram_parameter("wprojT", [128, 2, 256], BF16, isOutput=False).ap()
    maskc = nc.declare_dram_parameter("maskc", [128, 256], BF16, isOutput=False).ap()
    identc = nc.declare_dram_parameter("identc", [128, 128], BF16, isOutput=False).ap()
    y = nc.declare_dram_parameter("y", [img, ns, 3, 128, C], BF16, isOutput=True).ap()

    n = img * ns

    with tile.TileContext(nc) as tc:
        with (
            tc.tile_pool(name="const", bufs=1) as constp,
            tc.tile_pool(name="sb", bufs=1) as sb,
            tc.tile_pool(name="ps", bufs=1, space="PSUM") as ps,
        ):
            cst = {}
            for key, shape, src in (("wq", [128, 2, 768], wqkvT),
                                    ("wp", [128, 2, 256], wprojT),
                                    ("mask", [128, 256], maskc),
                                    ("id", [128, 128], identc)):
                cst[key] = constp.tile(shape, BF16, tag=key, name=key)
                nc.sync.dma_start(out=cst[key][:], in_=src[:])

            # v+ones tiles: 4 rotating buffers, ones columns set once
            v1_bufs = []
            for i in range(4):
                v1 = constp.tile([128, 3, 260], BF16, tag=f"v1_{i}")
                if "fullmemset" in _DBG:
                    nc.vector.memset(v1[:], 1.0)
                else:
                    oc = v1[:, :, 64:65]
                    nc.vector.memset(_ap(oc, [oc.ap[0], oc.ap[1], [65, 4]]), 1.0)
                v1_bufs.append(v1)

            # out staging: 3 rotating buffers, zeroed once so DRAM padding
            # rows read initialized data
            out_bufs = []
            for i in range(3):
                ob = constp.tile([128, 3, C], BF16, tag=f"ob_{i}")
                nc.vector.memset(ob[:], 0.0)
                out_bufs.append(ob)

            def _all():
                st = {}
                for t in range(n + 2):
                    if 1 <= t <= n and "stop1" not in _DBG:
                        _s2(nc, sb, ps, cst, st[t - 1])
                    if t < n:
                        st[t] = {"v1": v1_bufs[t % 4], "ob": out_bufs[t % 3],
                                 "img": t // ns, "w": t % ns}
                        _s1(nc, sb, ps, cst, x, st[t])
                    if t >= 2 and not ({"stop1", "stop2"} & _DBG):
                        _s3(nc, sb, ps, cst, y, st[t - 2])
                        del st[t - 2]

            if reps == 1:
                _all()
            else:
                with tc.For_i(0, reps, 1):
                    _all()
    return nc


def _s1(nc, sb, ps, cst, x, st):
    """DMA in, x^T, q^T/k^T, v(+ones layout). Produces qT, kT, v1."""
    img, w, v1_sb = st["img"], st["w"], st["v1"]
    wq_sb, ident_sb = cst["wq"], cst["id"]

    x_sb = sb.tile([128, 3, C], BF16, tag="x", bufs=3)
    if "blockdma" in _DBG:
        for b in range(3):
            nc.sync.dma_start(out=x_sb[:, b, :], in_=x[img, w, b])
    else:
        nc.sync.dma_start(out=x_sb[:], in_=x[img, w].rearrange("b p c -> p b c"))

    xT_ps = ps.tile([128, 2, 288], BF16, tag="tp", bufs=2)
    for b, kb in enumerate(KBS):
        for cc in range(2):
            nc.tensor.transpose(
                out=xT_ps[:, cc, 126 * b : 126 * b + kb],
                in_=x_sb[0:kb, b, 128 * cc : 128 * cc + 128],
                identity=ident_sb[0:kb, 0:kb],
            )
    xT_sb = sb.tile([128, 2, 288], BF16, tag="xTs", bufs=2)
    nc.vector.tensor_copy(out=xT_sb[:], in_=xT_ps[:])

    qT_sb = sb.tile([128, 2, 288], BF16, tag="qTs", bufs=3)
    kT_sb = sb.tile([128, 2, 288], BF16, tag="kTs", bufs=3)
    for base, dst, eng in ((0, qT_sb, "v"), (256, kT_sb, "s")):
        for mc in range(2):
            qp = ps.tile([128, 288], F32, tag="mm1", bufs=2)
            for kc in range(2):
                nc.tensor.matmul(
                    out=qp[:],
                    lhsT=wq_sb[:, kc, base + 128 * mc : base + 128 * mc + 128],
                    rhs=xT_sb[:, kc, :],
                    start=(kc == 0),
                    stop=(kc == 1),
                )
            if eng == "v":
                nc.vector.tensor_copy(out=dst[:, mc, :], in_=qp[:])
            else:
                nc.scalar.copy(out=dst[:, mc, :], in_=qp[:])

    for b, kb in enumerate(KBS):
        vp = ps.tile([128, 288], F32, tag="mm1", bufs=2)
        for kc in range(2):
            nc.tensor.matmul(
                out=vp[0:kb, 0:256],
                lhsT=xT_sb[:, kc, 126 * b : 126 * b + kb],
                rhs=wq_sb[:, kc, 512:768],
                start=(kc == 0),
                stop=(kc == 1),
            )
        vdst = v1_sb[0:kb, b, 0:260]
        nc.scalar.copy(
            out=_ap(vdst, [vdst.ap[0], [65, 4], [1, 64]]),
            in_=vp[0:kb, 0:256],
        )
    st["qT"], st["kT"] = qT_sb, kT_sb


def _s2(nc, sb, ps, cst, st):
    """QK^T logits, exp, window mask. Produces expm (masked)."""
    qT_sb, kT_sb = st["qT"], st["kT"]
    mask_sb = cst["mask"]

    expm = sb.tile([128, 12, 128], BF16, tag="expm", bufs=4)
    for b, kb in enumerate(KBS):
        # heads grouped by PE row-group: PSUM bank hh holds heads (hh, 2+hh);
        # MMs from different 64-row groups must not share a PSUM bank.
        aL = ps.tile([128, 2, 512], F32, tag="aL2", bufs=2)
        for h in range(4):
            mc, hh = h // 2, h % 2
            p0 = 64 * hh
            nc.tensor.matmul(
                out=aL[0:kb, hh, 128 * mc : 128 * mc + kb],
                lhsT=kT_sb[p0 : p0 + 64, mc, 126 * b : 126 * b + kb],
                rhs=qT_sb[p0 : p0 + 64, mc, 126 * b : 126 * b + kb],
                start=True,
                stop=True,
            )
        # expm slot order within block: h0, h2, h1, h3
        a0 = aL[0:kb, 0, 0:128]
        ein = _ap(a0, [a0.ap[0], [512, 2], [128, 2], [1, kb]])
        nc.scalar.activation(
            out=expm[0:kb, 4 * b : 4 * b + 4, 0:kb], in_=ein,
            func=mybir.ActivationFunctionType.Exp, scale=SCALE)

    eng = nc.vector if "dvemask" in _DBG else nc.gpsimd
    m = mask_sb[0:126, 0:126]
    eng.tensor_mul(
        out=expm[0:126, 0:8, 0:126], in0=expm[0:126, 0:8, 0:126],
        in1=_ap(m, [m.ap[0], [0, 8], m.ap[1]]))
    m2 = mask_sb[0:36, 128 : 128 + 36]
    eng.tensor_mul(
        out=expm[0:36, 8:12, 0:36], in0=expm[0:36, 8:12, 0:36],
        in1=_ap(m2, [m2.ap[0], [0, 4], m2.ap[1]]))
    st["expm"] = expm


def _s3(nc, sb, ps, cst, y, st):
    """AV(+denominator), normalize, ao^T, proj, DMA out."""
    img, w, v1_sb, expm = st["img"], st["w"], st["v1"], st["expm"]
    wp_sb, ident_sb = cst["wp"], cst["id"]

    r_sb = sb.tile([128, 12], F32, tag="rcp", bufs=2)
    ao_sb = sb.tile([128, 3, C], BF16, tag="aos", bufs=2)
    for b, kb in enumerate(KBS):
        avp2 = ps.tile([128, 2, 512], F32, tag="aL2", bufs=2)
        avp = avp2[:, 0, 0:272]
        for s in range(4):
            h = (0, 2, 1, 3)[s]  # expm slot s holds head h
            nc.tensor.matmul(
                out=avp[0:kb, 65 * h : 65 * h + 65],
                lhsT=expm[0:kb, 4 * b + s, 0:kb],
                rhs=v1_sb[0:kb, b, 65 * h : 65 * h + 65],
                start=True,
                stop=True,
            )
        dn = avp[0:kb, 64:65]
        if "norecip" in _DBG:
            nc.vector.tensor_copy(
                out=r_sb[0:kb, 4 * b : 4 * b + 4],
                in_=_ap(dn, [dn.ap[0], [65, 4]]))
        else:
            nc.vector.reciprocal_approx_fast(
                out=r_sb[0:kb, 4 * b : 4 * b + 4],
                in_=_ap(dn, [dn.ap[0], [65, 4]]))
        rr = r_sb[0:kb, 4 * b : 4 * b + 4]
        av0 = avp[0:kb, 0:260]
        if "nonorm" in _DBG:
            nc.vector.tensor_copy(
                out=ao_sb[0:kb, b, 0:256].rearrange("p (h d) -> p h d", h=4),
                in_=_ap(av0, [av0.ap[0], [65, 4], [1, 64]]))
        else:
            nc.vector.tensor_mul(
                out=ao_sb[0:kb, b, 0:256].rearrange("p (h d) -> p h d", h=4),
                in0=_ap(av0, [av0.ap[0], [65, 4], [1, 64]]),
                in1=_ap(rr, [rr.ap[0], [1, 4], [0, 64]]),
            )

    aoT_ps = ps.tile([128, 2, 288], BF16, tag="tp", bufs=2)
    for b, kb in enumerate(KBS):
        for cc in range(2):
            nc.tensor.transpose(
                out=aoT_ps[:, cc, 126 * b : 126 * b + kb],
                in_=ao_sb[0:kb, b, 128 * cc : 128 * cc + 128],
                identity=ident_sb[0:kb, 0:kb],
            )
    aoT_sb = sb.tile([128, 2, 288], BF16, tag="aoTs", bufs=2)
    nc.vector.tensor_copy(out=aoT_sb[:], in_=aoT_ps[:])

    out_sb = st["ob"]
    for b, kb in enumerate(KBS):
        op = ps.tile([128, 288], F32, tag="mm1", bufs=2)
        for T in range(2):
            nc.tensor.matmul(
                out=op[0:kb, 0:256],
                lhsT=aoT_sb[:, T, 126 * b : 126 * b + kb],
                rhs=wp_sb[:, T, :],
                start=(T == 0),
                stop=(T == 1),
            )
        if b == 2:
            nc.scalar.copy(out=out_sb[0:kb, b, :], in_=op[0:kb, 0:256])
        else:
            nc.vector.tensor_copy(out=out_sb[0:kb, b, :], in_=op[0:kb, 0:256])
    nc.sync.dma_start(out=y[img, w].rearrange("b p c -> p b c"), in_=out_sb[:])


def _make_consts():
    bf16 = ml_dtypes.bfloat16
    mask = np.zeros((128, 256), np.float32)
    for p in range(126):
        for q in range(126):
            if (p % 42) // 3 == (q % 42) // 3:
                mask[p, q] = 1.0
    for p in range(36):
        for q in range(36):
            if (p % 12) // 3 == (q % 12) // 3:
                mask[p, 128 + q] = 1.0
    return {
        "maskc": mask.astype(bf16),
        "identc": np.eye(128, dtype=np.float32).astype(bf16),
    }


_NC_CACHE = {}


def _get_nc():
    if "nc" not in _NC_CACHE:
        nc = bacc.Bacc("TRN2", target_bir_lowering=False, debug=False,
                       num_devices=NCORES)
        _build(nc)
        nc.compile()
        _NC_CACHE["nc"] = nc
    return _NC_CACHE["nc"]


def _permute_x(x):
    """[B, 9216, 256] f32 -> [B, 32, 3, 128, 256] bf16, window-row blocks."""
    bf16 = ml_dtypes.bfloat16
    xs = np.asarray(x, np.float32).reshape(B, S, 3, GRID, C)
    xp = np.zeros((B, S, 3, 128, C), dtype=bf16)
    for b, (wc0, nb) in enumerate(BLOCKS):
        cols = xs[:, :, :, 3 * wc0 : 3 * (wc0 + nb), :]  # [B,S,3,3nb,C]
        xp[:, :, b, : 9 * nb, :] = cols.reshape(B, S, 9 * nb, C).astype(bf16)
    return xp


def _unpermute_y(yp):
    """[B, 32, 3, 128, 256] bf16 -> [B, 9216, 256] f32."""
    ys = np.empty((B, S, 3, GRID, C), np.float32)
    for b, (wc0, nb) in enumerate(BLOCKS):
        blk = yp[:, :, b, : 9 * nb, :].astype(np.float32)
        ys[:, :, :, 3 * wc0 : 3 * (wc0 + nb), :] = blk.reshape(B, S, 3, 3 * nb, C)
    return ys.reshape(B, NT, C)


def _in_maps(x, Wqkv, Wproj):
    bf16 = ml_dtypes.bfloat16
    consts = _make_consts()
    consts["wqkvT"] = np.ascontiguousarray(
        np.asarray(Wqkv, np.float32).T.reshape(2, 128, 768).transpose(1, 0, 2)
    ).astype(bf16)
    consts["wprojT"] = np.ascontiguousarray(
        np.asarray(Wproj, np.float32).T.reshape(2, 128, 256).transpose(1, 0, 2)
    ).astype(bf16)
    xp = _permute_x(x)
    return [{"x": xp[IMG * c : IMG * c + IMG], **consts} for c in range(NCORES)]


def kernel(x, Wqkv, Wproj, H, W):
    assert int(H) == GRID and int(W) == GRID
    nc = _get_nc()
    res = run_bass_kernel_spmd(nc, _in_maps(x, Wqkv, Wproj), list(range(NCORES)))
    yp = np.concatenate([res.results[c]["y"] for c in range(NCORES)], axis=0)
    return _unpermute_y(yp)
